# revision 43
# baseline (speedup 1.0000x reference)
"""Trainium2 Bass kernel for nn_NodeModel (GNN message passing + external
attention + MLP), SPMD across 8 NeuronCores.

Sharding: nodes (and their incoming edges) are partitioned by destination-node
range across the 8 cores; small params are replicated. Host pre-sorts edges by
destination 128-node window; on-device segment_sum is one bf16 matmul per
128-edge chunk producing aggT [HID, nodes] directly (edge features stationary,
one-hot edge->node selection moving), accumulated in PSUM.

The whole node phase runs in the transposed domain with no on-device cat
transposes: x and u_b arrive from the host already transposed per window
(xT [128 feat, nodes], uT [64 feat, nodes], bf16), so the fused matmul
  [scores | h_partial | s1] = catT.T @ [Mkg | W1g | ones]
consumes host tiles + the aggT PSUM copy as its three K chunks. LayerNorm is
never materialized: a rank-1 PSUM correction (s1 x -colsum(M)/320) centers,
per-partition rstd rides the Exp/Relu activation scales, and 1/(ssum*rstd)
rides the softmax normalize. Variance: host ships 320*sumsq(x|u) per node;
the device adds the agg part via one bf16 square + a ones-matmul column.

DMA is mega-batched: edges / xT|uT / out move in 7 window-groups of 7 windows
(one descriptor-dense DMA each); edge destination lanes are one resident DMA.
"""

import sys

if "/opt/trn_rl_repo" not in sys.path:
    sys.path.insert(0, "/opt/trn_rl_repo")

import numpy as np

N, E, V_IN, HID, U_IN, B, MEM = 50000, 800000, 128, 128, 64, 64, 128
CAT = V_IN + HID + U_IN  # 320
ALPHA = 0.5
EPS = 1e-5
NCORES = 8
P = 128
N_LOC = N // NCORES        # 6250 nodes per core
NW = (N_LOC + P - 1) // P  # 49 windows of 128 nodes
N_PAD = NW * P             # 6272
WPG = 7                    # windows per DMA group
NG = NW // WPG             # 7 groups
MWC = 2 * P + 1            # mw columns: [Mkg | W1g | ones]
SSQC = MWC                 # ssq column index in ps_sh
PSW = MWC + 1              # ps_sh width


# ---------------------------------------------------------------------------
# Workarounds for this container's walrus: at most ONE sync wait per
# instruction is encodable. Tile's scheduler emits multi-waits; split them
# onto same-engine NoOps. Same for the TileContext exit drain.
# ---------------------------------------------------------------------------

def _patched_drain_and_barrier(self, tick_clock, wait_clock):
    from concourse.vector_clock import ScopedClock, VectorClock

    nc = self.nc
    gvc = tick_clock.global_clock
    nprocs = len(gvc)
    for proc in range(nprocs):
        tick = gvc[proc]
        if tick <= 0:
            continue
        one = VectorClock([0] * nprocs)
        one.require_at_least(proc, tick)
        inst = nc.sync.drain()
        wait_clock.add_sem_waits(inst.ins, ScopedClock({None: one}))
    nc.sync.drain()
    nc.all_engine_barrier()
    assert self.sems is not None
    popped = nc._tile_sem_poison_stack.pop()
    assert popped is self._sem_poison
    nc.clear_and_free_semaphores(list(self.sems.allocated().values()))
    nc.all_engine_barrier()


def _split_multi_waits(nc):
    from concourse import mybir

    for f in nc.m.functions:
        for bb in f.blocks:
            out = []
            for inst in bb.instructions:
                si = inst.sync_info
                if si is not None and si.on_wait is not None and len(si.on_wait) > 1:
                    waits = list(si.on_wait)
                    for i, w in enumerate(waits[:-1]):
                        out.append(mybir.InstNoOp(
                            name=f"{inst.name}-wsplit{i}",
                            engine=inst.engine,
                            sync_info=mybir.SyncInfo(on_wait=[w], on_update=[]),
                        ))
                    si.on_wait = waits[-1:]
                out.append(inst)
            bb.instructions[:] = out


_patch_applied = False


def _apply_patches():
    global _patch_applied
    if _patch_applied:
        return
    import concourse.tile as tile

    tile.TileContext._drain_and_barrier = _patched_drain_and_barrier
    _patch_applied = True


# ---------------------------------------------------------------------------
# Bass module builder. Kernel structure depends only on the per-window chunk
# counts C (shared across cores), so cache on that.
# ---------------------------------------------------------------------------

_nc_cache = {}
DEBUG = False


def _build(key, split_waits=True):
    """key: (C, sb_zero, b2_zero); C = per-window 128-edge chunk counts."""
    import concourse.bass as bass
    import concourse.tile as tile
    from concourse import mybir

    C, sb_zero, b2_zero = key
    _apply_patches()
    f32 = mybir.dt.float32
    bf16 = mybir.dt.bfloat16
    Ctot = sum(C)
    qoff = []
    _q = 0
    for w in range(NW):
        qoff.append(_q)
        _q += C[w]
    CG = [sum(C[g * WPG:(g + 1) * WPG]) for g in range(NG)]
    goff = [qoff[g * WPG] for g in range(NG)]
    CGmax = max(CG)

    nc = bass.Bass()
    d_ea = nc.dram_tensor("ea", [Ctot * P * HID], bf16, kind="ExternalInput")
    d_dstl = nc.dram_tensor("dstl", [P * Ctot], f32, kind="ExternalInput")
    d_xtx = nc.dram_tensor("xtx", [NG * P * WPG * P], bf16, kind="ExternalInput")
    d_xtu = nc.dram_tensor("xtu", [NG * U_IN * WPG * P], bf16,
                           kind="ExternalInput")
    d_sxq = nc.dram_tensor("sxq", [NG * P * WPG], f32, kind="ExternalInput")
    d_mw = nc.dram_tensor("mw", [P, 3 * MWC], bf16, kind="ExternalInput")
    d_kfix = nc.dram_tensor("kfix", [1, 2 * P], bf16, kind="ExternalInput")
    d_mv1 = nc.dram_tensor("mv1", [MEM, HID], bf16, kind="ExternalInput")
    d_w2 = nc.dram_tensor("w2", [HID, HID], bf16, kind="ExternalInput")
    d_iota = nc.dram_tensor("iota", [P, P], bf16, kind="ExternalInput")
    d_idb = nc.dram_tensor("identb", [P, P], bf16, kind="ExternalInput")
    if not sb_zero:
        d_sb = nc.dram_tensor("sb", [1, MEM], f32, kind="ExternalInput")
    if not b2_zero:
        d_b2 = nc.dram_tensor("b2", [1, HID], f32, kind="ExternalInput")
    d_out = nc.dram_tensor("out", [NG * P * WPG * HID], f32, kind="ExternalOutput")
    if DEBUG:
        _dbgdt = {"aggT": bf16, "sqa": bf16, "s1r": bf16, "sd": f32,
                  "rstd": f32, "pt": bf16, "h": bf16, "sc": f32,
                  "nm": f32, "ss": f32, "sc0": f32}
        d_dbg = {n: nc.dram_tensor("dbg_" + n, [P, P], dt,
                                   kind="ExternalOutput")
                 for n, dt in _dbgdt.items()}

    def dbg_dump(st, n, tile, rows=P):
        if DEBUG and st["g"] == 0 and st["wl"] == 0:
            nc.sync.dma_start(out=d_dbg[n][:rows, :tile.shape[-1]],
                              in_=tile)

    with tile.TileContext(nc) as tc:
        with (
            tc.tile_pool(name="const", bufs=1) as cpool,
            tc.tile_pool(name="edges", bufs=3) as epool,
            tc.tile_pool(name="xg", bufs=3) as xgpool,
            tc.tile_pool(name="ytg", bufs=3) as ypool,
            tc.tile_pool(name="oh", bufs=40) as ohpool,
            tc.tile_pool(name="small", bufs=12) as spool,
            tc.tile_pool(name="work", bufs=6) as wpool,
            tc.tile_pool(name="agg_ps", bufs=2, space="PSUM") as aggps,
            tc.tile_pool(name="tr_ps", bufs=3, space="PSUM") as trps,
            tc.tile_pool(name="mm_ps", bufs=3, space="PSUM") as mmps,
        ):
            # ---- constants ----
            t_mw = cpool.tile([P, 3, MWC], bf16)
            nc.sync.dma_start(
                out=t_mw[:].rearrange("p a b -> p (a b)"), in_=d_mw[:, :])
            t_kfix = cpool.tile([1, 2 * P], bf16)
            nc.sync.dma_start(out=t_kfix[:1], in_=d_kfix[:, :])
            t_mv1 = cpool.tile([MEM, HID], bf16)
            nc.sync.dma_start(out=t_mv1[:], in_=d_mv1[:, :])
            t_w2 = cpool.tile([P, P], bf16)
            nc.sync.dma_start(out=t_w2[:], in_=d_w2[:, :])
            t_iota = cpool.tile([P, P], bf16)
            nc.sync.dma_start(out=t_iota[:], in_=d_iota[:, :])
            t_idb = cpool.tile([P, P], bf16)
            nc.sync.dma_start(out=t_idb[:], in_=d_idb[:, :])
            t_dstl = cpool.tile([P, Ctot], f32)
            nc.sync.dma_start(
                out=t_dstl[:], in_=d_dstl[:].rearrange("(p f) -> p f", p=P))
            if not sb_zero:
                t_sb = cpool.tile([1, MEM], f32)
                nc.sync.dma_start(out=t_sb[:1], in_=d_sb[:, :])
            if not b2_zero:
                t_b2 = cpool.tile([1, HID], f32)
                nc.sync.dma_start(out=t_b2[:1], in_=d_b2[:, :])
            t_ones1 = cpool.tile([P, 1], bf16)
            nc.vector.memset(t_ones1[:], 1.0)
            if not b2_zero:
                t_onesr = cpool.tile([1, P], bf16)
                nc.vector.memset(t_onesr[:1], 1.0)

            A = mybir.AluOpType
            AF = mybir.ActivationFunctionType

            def emit_seg(w, e_tile, ecol, mid_cb=None, late_cb=None):
                """aggT[HID, nodes] = sum_c e_chunk.T @ onehot for window w."""
                ps_agg = aggps.tile([P, P], f32)
                mid = C[w] // 2
                late = (3 * C[w]) // 4
                for c in range(C[w]):
                    if c == mid and mid_cb is not None:
                        mid_cb()
                    if c == late and late_cb is not None:
                        late_cb()
                    q = qoff[w] + c
                    oh = ohpool.tile([P, P], bf16, tag="oh")
                    nc.vector.tensor_scalar(
                        out=oh[:], in0=t_iota[:],
                        scalar1=t_dstl[:, q:q + 1], scalar2=None,
                        op0=A.is_equal,
                    )
                    nc.tensor.matmul(
                        ps_agg[:], lhsT=e_tile[:, ecol + c, :], rhs=oh[:],
                        start=(c == 0), stop=(c == C[w] - 1))
                return ps_agg

            def aggcopy(st):
                aggT = wpool.tile([P, P], bf16, tag="aggT")
                nc.scalar.copy(out=aggT[:], in_=st["ps_agg"][:])
                st["aggT"] = aggT
                dbg_dump(st, "aggT", aggT[:])

            def sqs(st):
                sqa = wpool.tile([P, P], bf16, tag="sqa")
                nc.gpsimd.tensor_tensor(out=sqa[:], in0=st["aggT"][:],
                                        in1=st["aggT"][:], op=A.mult)
                st["sqa"] = sqa
                dbg_dump(st, "sqa", sqa[:])

            def s1x(st):
                """fused mm on [xT|uT|aggT] + ssq col + s1 cols + var + sd."""
                wl = st["wl"]
                xtx_t, xtu_t, sxq_t = st["xtx"], st["xtu"], st["sxq"]
                aggT = st["aggT"]
                sqa = st["sqa"]
                ps_sh = mmps.tile([P, PSW], f32, tag="mm")
                ch = [(0, xtx_t[:, wl, :]), (1, xtu_t[:U_IN, wl, :]),
                      (2, aggT[:])]
                for j, lhsT in ch:
                    nc.tensor.matmul(ps_sh[:, 0:MWC],
                                     lhsT=lhsT,
                                     rhs=t_mw[:lhsT.partition_size(), j, :],
                                     start=(j == 0), stop=(j == 2))
                st["ps_sh"] = ps_sh
                # s1 row directly: ones.T @ catT_j accumulated into [1, nodes]
                ps_s1r = trps.tile([P, 2 * P], f32, tag="tr")
                for j, lhsT in ch:
                    nc.tensor.matmul(ps_s1r[:1, 0:P],
                                     lhsT=t_ones1[:lhsT.partition_size(), :],
                                     rhs=lhsT,
                                     start=(j == 0), stop=(j == 2))
                s1r = spool.tile([1, P], bf16, tag="s1r")
                nc.scalar.copy(out=s1r[:1], in_=ps_s1r[:1, 0:P])
                st["s1r"] = s1r
                st["ps_s1r"] = ps_s1r
                dbg_dump(st, "s1r", s1r[:1], rows=1)
                if DEBUG and st["g"] == 0 and st["wl"] == 0:
                    sc0 = spool.tile([P, P], f32, tag="sc0d")
                    nc.scalar.copy(out=sc0[:], in_=ps_sh[:, 0:P])
                    dbg_dump(st, "sc0", sc0[:])
                if not sb_zero:
                    sdc = spool.tile([P, 1], bf16, tag="sdc")
                    nc.gpsimd.tensor_copy(out=sdc[:], in_=sd[:])
                    ptr2 = trps.tile([P, P], bf16, tag="tr")
                    nc.tensor.transpose(out=ptr2[:1, :], in_=sdc[:, :],
                                        identity=t_idb[:])
                    sdr = spool.tile([1, P], bf16, tag="sdr")
                    nc.scalar.copy(out=sdr[:1], in_=ptr2[:1, :])
                    sbb = spool.tile([1, P], bf16, tag="sbb")
                    nc.gpsimd.tensor_copy(out=sbb[:1], in_=t_sb[:1])
                    st["sdr"], st["sbb"] = sdr, sbb

            def rank1(st):
                ps_sh = st["ps_sh"]
                nc.tensor.matmul(ps_sh[:, 0:2 * P], lhsT=st["s1r"][:1, :],
                                 rhs=t_kfix[:1, :], start=False, stop=sb_zero,
                                 skip_group_check=True)
                if not sb_zero:
                    nc.tensor.matmul(ps_sh[:, 0:P], lhsT=st["sdr"][:1, :],
                                     rhs=st["sbb"][:1, :], start=False,
                                     stop=True, skip_group_check=True)
                ps_s1r = st["ps_s1r"]
                nc.tensor.matmul(ps_s1r[:, P:P + 1],
                                 lhsT=st["sqa"][:], rhs=t_ones1[:, :],
                                 start=True, stop=True,
                                 skip_group_check=True)
                t2 = spool.tile([P, 1], f32, tag="t2")
                nc.scalar.activation(out=t2[:], in_=ps_sh[:, 2 * P:2 * P + 1],
                                     func=AF.Square)
                r = spool.tile([P, 1], f32, tag="r")
                nc.vector.tensor_scalar(
                    out=r[:], in0=ps_s1r[:, P:P + 1], scalar1=320.0,
                    scalar2=t2[:, :1], op0=A.mult, op1=A.subtract)
                # bias sxq = ssq(x|u)/320 + eps  ->  sd = sqrt(var + eps)
                sd = spool.tile([P, 1], f32, tag="sd")
                nc.scalar.activation(out=sd[:], in_=r[:], func=AF.Sqrt,
                                     bias=st["sxq"][:, st["wl"]:st["wl"] + 1],
                                     scale=1.0 / (320.0 * 320.0))
                st["sd"] = sd
                dbg_dump(st, "sd", sd[:])

            def s2(st):
                """rstd; (sb row); scores max; exp with rstd folded."""
                ps_sh = st["ps_sh"]
                if DEBUG and st["g"] == 0 and st["wl"] == 0:
                    scd = spool.tile([P, P], f32, tag="scd")
                    nc.scalar.copy(out=scd[:], in_=ps_sh[:, 0:P])
                    dbg_dump(st, "sc", scd[:])
                nm = spool.tile([P, 1], f32, tag="nm")
                nc.vector.tensor_reduce(out=nm[:], in_=ps_sh[:, 0:P],
                                        axis=mybir.AxisListType.X,
                                        op=A.max, negate=True)
                rstd = spool.tile([P, 1], f32, tag="rstd")
                nc.vector.reciprocal(out=rstd[:], in_=st["sd"][:])
                st["rstd"] = rstd
                dbg_dump(st, "rstd", rstd[:])
                nm2 = spool.tile([P, 1], f32, tag="nm2")
                nc.gpsimd.tensor_scalar(
                    out=nm2[:], in0=nm[:], scalar1=rstd[:, :1], scalar2=None,
                    op0=A.mult)
                pt = wpool.tile([P, MEM], bf16, tag="pt")
                ssum = spool.tile([P, 1], f32, tag="ss")
                nc.scalar.activation(out=pt[:], in_=ps_sh[:, 0:P],
                                     func=AF.Exp,
                                     bias=nm2[:, :1], scale=rstd[:, :1],
                                     accum_out=ssum[:, :1])
                st["pt"], st["ssum"] = pt, ssum
                dbg_dump(st, "pt", pt[:])
                dbg_dump(st, "nm", nm[:])
                dbg_dump(st, "ss", ssum[:])

            def s3(st):
                """softmax normalize (1/(ssum*rstd)), attn mm, relu."""
                ps_sh, rstd, pt = st["ps_sh"], st["rstd"], st["pt"]
                rsin = spool.tile([P, 1], f32, tag="rsin")
                nc.gpsimd.tensor_scalar(
                    out=rsin[:], in0=st["ssum"][:], scalar1=rstd[:, :1],
                    scalar2=None, op0=A.mult)
                rs = spool.tile([P, 1], f32, tag="rs")
                nc.vector.reciprocal(out=rs[:], in_=rsin[:])
                nc.vector.tensor_scalar(out=pt[:], in0=pt[:],
                                        scalar1=rs[:, :1], scalar2=None,
                                        op0=A.mult)
                ptr3 = trps.tile([P, P], bf16, tag="tr")
                nc.tensor.transpose(out=ptr3[:], in_=pt[:], identity=t_idb[:])
                aT = wpool.tile([P, P], bf16, tag="aT")
                nc.vector.tensor_copy(out=aT[:], in_=ptr3[:])
                nc.tensor.matmul(ps_sh[:, P:2 * P], lhsT=aT[:], rhs=t_mv1[:],
                                 start=False, stop=True, skip_group_check=True)
                h = wpool.tile([P, HID], bf16, tag="h")
                nc.scalar.activation(out=h[:], in_=ps_sh[:, P:2 * P],
                                     func=AF.Relu, scale=rstd[:, :1])
                st["h"] = h
                dbg_dump(st, "h", h[:])

            def s4(st):
                """hT -> W2 -> yt (+ group store)."""
                ptr4 = trps.tile([P, P], bf16, tag="tr")
                nc.tensor.transpose(out=ptr4[:], in_=st["h"][:],
                                    identity=t_idb[:])
                hT = wpool.tile([P, P], bf16, tag="hT")
                nc.scalar.copy(out=hT[:], in_=ptr4[:])
                ps_y = trps.tile([P, P], f32, tag="tr")
                nc.tensor.matmul(ps_y[:], lhsT=hT[:], rhs=t_w2[:],
                                 start=True, stop=b2_zero)
                if not b2_zero:
                    nc.tensor.matmul(ps_y[:], lhsT=t_onesr[:1, :],
                                     rhs=t_b2[:1, :], start=False, stop=True,
                                     skip_group_check=True)
                nc.scalar.copy(out=st["yt_t"][:, st["wl"], :], in_=ps_y[:])
                if st["wl"] == WPG - 1:
                    gp = st["g"]
                    nc.sync.dma_start(
                        out=d_out[gp * P * WPG * HID:
                                  (gp + 1) * P * WPG * HID].rearrange(
                            "(p f) -> p f", p=P),
                        in_=st["yt_t"][:].rearrange("p w u -> p (w u)"),
                    )

            # ---- group prefetch ----
            group_tiles = {}

            def prefetch(g):
                if g >= NG or g in group_tiles:
                    return
                e_tile = epool.tile([P, CGmax, HID], bf16, tag="ed")
                nc.sync.dma_start(
                    out=e_tile[:, :CG[g], :],
                    in_=d_ea[goff[g] * P * HID:
                             (goff[g] + CG[g]) * P * HID].rearrange(
                        "(p f) -> p f", p=P),
                )
                xtx_t = xgpool.tile([P, WPG, P], bf16, tag="xtx")
                nc.sync.dma_start(
                    out=xtx_t[:],
                    in_=d_xtx[g * P * WPG * P:(g + 1) * P * WPG * P].rearrange(
                        "(p f) -> p f", p=P).rearrange(
                        "p (w u) -> p w u", w=WPG),
                )
                xtu_t = xgpool.tile([U_IN, WPG, P], bf16, tag="xtu")
                nc.sync.dma_start(
                    out=xtu_t[:],
                    in_=d_xtu[g * U_IN * WPG * P:
                              (g + 1) * U_IN * WPG * P].rearrange(
                        "(p f) -> p f", p=U_IN).rearrange(
                        "p (w u) -> p w u", w=WPG),
                )
                sxq_t = xgpool.tile([P, WPG], f32, tag="sxq")
                nc.sync.dma_start(
                    out=sxq_t[:],
                    in_=d_sxq[g * P * WPG:(g + 1) * P * WPG].rearrange(
                        "(p f) -> p f", p=P),
                )
                yt_t = ypool.tile([P, WPG, HID], f32, tag="yt")
                group_tiles[g] = (e_tile, xtx_t, xtu_t, sxq_t, yt_t)

            # ---- main loop: staged, skewed; fused matmuls hide inside
            # the one-hot stream (mid_cb), rank-1 fix a bit later (late_cb)
            states = {}
            prefetch(0)

            for i in range(NW + 4):
                if 0 <= i - 1 < NW:
                    aggcopy(states[i - 1])
                if 0 <= i - 4 < NW:
                    s4(states[i - 4])
                if 0 <= i - 1 < NW:
                    sqs(states[i - 1])
                if 0 <= i - 2 < NW:
                    s2(states[i - 2])
                if i < NW:
                    g, wl = divmod(i, WPG)
                    if wl == 0:
                        prefetch(g + 1)
                    e_tile, xtx_t, xtu_t, sxq_t, yt_t = group_tiles[g]
                    st = {"g": g, "wl": wl, "xtx": xtx_t, "xtu": xtu_t,
                          "sxq": sxq_t, "yt_t": yt_t}
                    states[i] = st
                    mid = (lambda j: (lambda: s1x(states[j])))(i - 1) \
                        if 0 <= i - 1 < NW else None
                    late = (lambda j: (lambda: rank1(states[j])))(i - 1) \
                        if 0 <= i - 1 < NW else None
                    st["ps_agg"] = emit_seg(i, e_tile, qoff[i] - goff[g],
                                            mid_cb=mid, late_cb=late)
                else:
                    if 0 <= i - 1 < NW:
                        s1x(states[i - 1])
                        rank1(states[i - 1])
                if 0 <= i - 3 < NW:
                    s3(states[i - 3])
                if 0 <= i - 4 < NW:
                    del states[i - 4]

    if split_waits:
        _split_multi_waits(nc)
    return nc


def _prepare(x, edge_index, edge_attr, u, batch, Mk, Mv, ln_gamma, ln_beta,
             W1, b1, W2, b2):
    """Host-side sharding / packing. Returns (key, in_maps)."""
    import ml_dtypes
    bf = ml_dtypes.bfloat16

    x = np.asarray(x, dtype=np.float32)
    edge_attr = np.asarray(edge_attr, dtype=np.float32)
    u = np.asarray(u, dtype=np.float32)
    Mk = np.asarray(Mk, dtype=np.float32)
    Mv = np.asarray(Mv, dtype=np.float32)
    g = np.asarray(ln_gamma, dtype=np.float32)
    be = np.asarray(ln_beta, dtype=np.float32)
    W1 = np.asarray(W1, dtype=np.float32)
    b1 = np.asarray(b1, dtype=np.float32)
    W2 = np.asarray(W2, dtype=np.float32)
    b2 = np.asarray(b2, dtype=np.float32)
    dst = np.asarray(edge_index)[1].astype(np.int64)
    batch = np.asarray(batch).astype(np.int64)

    core_id = dst // N_LOC
    rem = dst - core_id * N_LOC
    w_id = rem >> 7
    loc = (rem & 127).astype(np.float32)
    sort_key = core_id * NW + w_id
    order = np.argsort(sort_key, kind="stable")
    counts = np.bincount(sort_key, minlength=NCORES * NW).reshape(NCORES, NW)
    C = np.maximum((counts.max(axis=0) + P - 1) // P, 1).astype(np.int64)
    Ctot = int(C.sum())
    qoff = np.concatenate([[0], np.cumsum(C[:-1])])

    starts = np.concatenate([[0], np.cumsum(counts.reshape(-1))])
    loc_sorted = loc[order]

    # per-core edge payload: per window, lane p holds rows {c*P+p}. Each
    # GROUP block is p-major within the group so it moves as ONE
    # [P, CG*HID] DMA; dstl is p-major globally (single resident DMA).
    CG = [int(C[g * WPG:(g + 1) * WPG].sum()) for g in range(NG)]
    goff = [int(qoff[g * WPG]) for g in range(NG)]
    ea_pad = np.zeros((NCORES, Ctot * P * HID), dtype=bf)
    dstl_t = np.full((NCORES, P, Ctot), -1.0, dtype=np.float32)
    for c in range(NCORES):
        gviews = [
            ea_pad[c][goff[g] * P * HID:(goff[g] + CG[g]) * P * HID]
            .reshape(P, CG[g], HID) for g in range(NG)
        ]
        for w in range(NW):
            k = c * NW + w
            s, e = starts[k], starts[k + 1]
            cnt = e - s
            Cw = int(C[w])
            blkf = np.zeros((Cw * P, HID), dtype=np.float32)
            blkf[:cnt] = edge_attr[order[s:e]]
            blk = blkf.astype(bf).reshape(Cw, P, HID).transpose(1, 0, 2)
            base = int(qoff[w])
            gw = w // WPG
            gviews[gw][:, base - goff[gw]:base - goff[gw] + Cw, :] = blk
            lb = np.full(Cw * P, -1.0, dtype=np.float32)
            lb[:cnt] = loc_sorted[s:e]
            dstl_t[c, :, base:base + Cw] = lb.reshape(Cw, P).T

    # transposed x / u_b in bf16, group-lane-major: [C, NG, feat, WPG, nodes]
    u_b = u[batch]
    xb = x.astype(bf)
    ub = u_b.astype(bf)
    xpad = np.zeros((NCORES, N_PAD, V_IN), dtype=bf)
    upad = np.zeros((NCORES, N_PAD, U_IN), dtype=bf)
    xpad[:, :N_LOC] = xb.reshape(NCORES, N_LOC, V_IN)
    upad[:, :N_LOC] = ub.reshape(NCORES, N_LOC, U_IN)
    # [C, NG, WPG, P(nodes), F] -> [C, NG, F, WPG, P]
    xtx = np.ascontiguousarray(
        xpad.reshape(NCORES, NG, WPG, P, V_IN).transpose(0, 1, 4, 2, 3)
    ).reshape(NCORES, -1)
    xtu = np.ascontiguousarray(
        upad.reshape(NCORES, NG, WPG, P, U_IN).transpose(0, 1, 4, 2, 3)
    ).reshape(NCORES, -1)
    # 320 * sumsq of the (bf16-rounded) x|u part per node: [C, NG, P, WPG]
    ssq_xu = (xpad.astype(np.float32) ** 2).sum(axis=2) + \
             (upad.astype(np.float32) ** 2).sum(axis=2)
    sxq = np.ascontiguousarray(
        (ssq_xu / 320.0 + EPS).reshape(NCORES, NG, WPG, P).transpose(0, 1, 3, 2)
    ).reshape(NCORES, -1).astype(np.float32)

    # weights with cat order [x | u | agg] (reference order is [x, agg, u])
    perm = np.concatenate([np.arange(0, V_IN),
                           np.arange(V_IN + HID, CAT),
                           np.arange(V_IN, V_IN + HID)])
    mkg = (Mk * g[None, :]).T[perm]                           # [CAT, MEM]
    w1g = ((1.0 - ALPHA) * g[:, None] * W1)[perm]             # [CAT, HID]
    mwf = np.concatenate([mkg, w1g, np.ones((CAT, 1), np.float32)], axis=1)
    mwb = mwf.astype(bf)                                      # [320, 257]
    mw3 = np.zeros((P, 3 * MWC), dtype=bf)
    KCH = [(0, 0, V_IN), (1, V_IN, U_IN), (2, V_IN + U_IN, HID)]
    for j, off, K in KCH:
        mw3[:K, j * MWC:(j + 1) * MWC] = mwb[off:off + K]
    kfix = (-(mwb[:, :2 * P].astype(np.float32).sum(axis=0)) / 320.0).astype(
        bf).reshape(1, 2 * P)

    b1p = (1.0 - ALPHA) * (be @ W1) + b1
    mv1 = (ALPHA * (Mv @ W1) + b1p[None, :]).astype(bf)       # [MEM, HID]
    sb = (Mk @ be).reshape(1, MEM)
    b2r = b2.reshape(1, HID)
    iota = np.tile(np.arange(P, dtype=np.float32), (P, 1)).astype(bf)
    identb = np.eye(P, dtype=np.float32).astype(bf)

    key = (tuple(int(v) for v in C),
           bool(np.all(sb == 0.0)), bool(np.all(b2r == 0.0)))

    in_maps = []
    for c in range(NCORES):
        m = {
            "ea": ea_pad[c], "dstl": dstl_t[c].reshape(-1),
            "xtx": xtx[c], "xtu": xtu[c], "sxq": sxq[c],
            "mw": mw3, "kfix": kfix, "mv1": mv1,
            "w2": W2.astype(bf),
            "iota": iota, "identb": identb,
        }
        if not key[1]:
            m["sb"] = sb
        if not key[2]:
            m["b2"] = b2r
        in_maps.append(m)
    return key, in_maps


def kernel(**inputs):
    from concourse import bass_utils

    key, in_maps = _prepare(**inputs)
    nc = _nc_cache.get(key)
    if nc is None:
        nc = _build(key)
        _nc_cache[key] = nc
    res = bass_utils.run_bass_kernel_spmd(nc, in_maps, core_ids=list(range(NCORES)))
    # un-permute [NG, P, WPG, HID] -> [N_PAD, HID]
    outs = []
    for r in res.results:
        o = r["out"].reshape(NG, P, WPG, HID).transpose(0, 2, 1, 3)
        outs.append(o.reshape(N_PAD, HID)[:N_LOC])
    return np.concatenate(outs, axis=0).astype(np.float32)
